# revision 1
# baseline (speedup 1.0000x reference)
"""Trainium2 Bass kernel for nn_DiffKS (differentiable Karplus-Strong).

Structure of the computation:
  y[t] = x[t] - sum_{j=0..5} vals[t,j] * y[t - 1 - z_l[t] - j]
with vals / z_l derived from spline-interpolated delay & coefficient
trajectories.  The feedback lag (1 + z_l + j) is always >= ~93 samples, so
128-sample chunks can be computed as dense banded matmuls against a
512-sample window of past output (4 ring columns of 128, partition-aligned)
plus a small within-chunk correction term.

8-core strategy (the recurrence is strictly sequential in time; there is a
single voice, so we parallelise over time segments using linearity):
  - split the 65536 samples into 8 segments of 8192 (one per NeuronCore)
  - phase B (parallel): every core runs its segment's chunked recurrence
    with basis+1 right-hand sides (basis = max feedback lag, ~427): unit
    "basis" initial-window columns + 1 particular column (the excitation
    with zero initial window).  This yields, per chunk, its response
    operator H_c (stored to DRAM), and per segment its transfer operator:
    final-window = T[:, :basis] @ initial-window + T[:, basis].
  - combine (host, tiny): chain the 8 transfer operators to get every
    segment's true initial window w_s (8 small matvecs).
  - apply (parallel): y[:, c] = H_c @ [w_s; 1] — one fused multiply+
    accumulate-reduce per chunk on the Vector engine, plus within-chunk
    correction fix-ups.

Performance notes: all-zero 128x128 weight blocks are skipped (shared SPMD
plans = per-position union across the 8 segments); each chunk's
within-chunk correction is algebraically folded into the weights of its
downstream readers on the host (_fold_corr), which removes the correction
matmul from the serial chunk-to-chunk dependency chain; DIFFKS_FASTB=1
optionally runs phase B matmuls in float32r (FP22 multiplies, ~1.25x
faster end-to-end, relative error 2.7e-4 instead of 1.6e-5).
"""

import os
import numpy as np

import concourse.bacc as bacc
import concourse.tile as tile
import concourse.mybir as mybir
from concourse.bass_utils import run_bass_kernel_spmd


def _ensure_ntff_hook():
    """The agent image's `antenv` stub lacks `axon_hooks`, which
    `run_bass_kernel_spmd(trace=True)` needs under axon for NTFF capture.
    Recreate the same ctypes-based hook `trn_agent_boot.trn_boot` would
    install on images where the module exists (see its section 6)."""
    try:
        from antenv.axon_hooks import get_axon_ntff_profile_hook  # noqa: F401
        return
    except ImportError:
        pass
    import contextlib
    import ctypes
    import sys
    import types

    so_path = "/opt/axon/libaxon_pjrt.so"
    if not os.path.exists(so_path):
        return
    lib = ctypes.CDLL(so_path)
    if not hasattr(lib, "axon_start_nrt_profile"):
        return
    lib.axon_start_nrt_profile.argtypes = [
        ctypes.POINTER(ctypes.c_int64), ctypes.c_size_t]
    lib.axon_start_nrt_profile.restype = ctypes.c_int64
    lib.axon_stop_nrt_profile.argtypes = [ctypes.c_char_p]
    lib.axon_stop_nrt_profile.restype = ctypes.c_int64

    @contextlib.contextmanager
    def _hook(output_dir, device_ids):
        import jax
        jax.devices()
        if device_ids:
            ids = (ctypes.c_int64 * len(device_ids))(*device_ids)
            rc = lib.axon_start_nrt_profile(ids, len(device_ids))
        else:
            rc = lib.axon_start_nrt_profile(None, 0)
        if rc != 0:
            raise RuntimeError(f"axon_start_nrt_profile rc={rc}")
        try:
            yield
        finally:
            n = lib.axon_stop_nrt_profile(str(output_dir).encode())
            if n <= 0:
                print(f"ntff profile: {n} file(s) written to {output_dir}",
                      file=sys.stderr)

    mod = types.ModuleType("antenv.axon_hooks")
    mod._hook = _hook
    mod.get_axon_ntff_profile_hook = lambda: _hook
    mod.set_axon_ntff_profile_hook = lambda h: setattr(mod, "_hook", h)
    import antenv
    antenv.axon_hooks = mod
    sys.modules["antenv.axon_hooks"] = mod


_ensure_ntff_hook()

F32 = mybir.dt.float32

N_SAMPLES = 65536
N_FRAMES = 64
L_ORDER = 5
CHUNK = 128
WIN = 512            # window length the chunk matmuls see (4 ring cols)
RING = 8             # ring columns in SBUF (power of two, >= 5)
CORR = 64            # within-chunk correction width (needs z_l >= 63)
BASIS = 448          # basis = window positions 64..511 (needs z_l <= 442)
NRHS = BASIS + 1
N_CORES = 8

# filled by kernel() with per-phase profiling results for the test harness
LAST_RESULTS = {}

# built bass programs, keyed by structure — repeated kernel() calls with the
# same inputs reuse the same program objects (and their compile caches)
_NC_CACHE = {}


# ----------------------------------------------------------------------------
# host-side preprocessing (input-independent spline matrix + tiny elementwise)
# ----------------------------------------------------------------------------

_SPLINE_CACHE = {}


def _spline_matrix(n_in, n_out):
    """Static [n_out, n_in] natural-cubic-spline interpolation matrix for
    uniform knots t_in=linspace(0,1,n_in) evaluated at linspace(0,1,n_out).
    Input-independent (depends only on the fixed shapes)."""
    key = (n_in, n_out)
    if key in _SPLINE_CACHE:
        return _SPLINE_CACHE[key]
    t_in = np.linspace(0.0, 1.0, n_in)
    t_out = np.linspace(0.0, 1.0, n_out)
    n = n_in
    h = t_in[1:] - t_in[:-1]
    R = np.zeros((n - 2, n))
    for i in range(n - 2):
        R[i, i] += 6.0 / h[i]
        R[i, i + 1] += -6.0 / h[i] - 6.0 / h[i + 1]
        R[i, i + 2] += 6.0 / h[i + 1]
    A = (
        np.diag(2.0 * (h[:-1] + h[1:]))
        + np.diag(h[1:-1], 1)
        + np.diag(h[1:-1], -1)
    )
    M = np.zeros((n, n))
    M[1:-1] = np.linalg.solve(A, R)          # second derivatives, linear in y
    idx = np.clip(np.searchsorted(t_in, t_out, side="right") - 1, 0, n - 2)
    dt = t_out - t_in[idx]
    S = np.zeros((n_out, n))
    eye = np.eye(n)
    for r in range(n_out):
        i = idx[r]
        b = (eye[i + 1] - eye[i]) / h[i] - h[i] * (2.0 * M[i] + M[i + 1]) / 6.0
        c = M[i] / 2.0
        d = (M[i + 1] - M[i]) / (6.0 * h[i])
        S[r] = eye[i] + b * dt[r] + c * dt[r] ** 2 + d * dt[r] ** 3
    S = S.astype(np.float32)
    _SPLINE_CACHE[key] = S
    return S


def _preprocess(delay, raw, exc, n_samples):
    sig = 1.0 / (1.0 + np.exp(-np.asarray(raw, np.float32)))
    coeff = sig / sig.sum(-1, keepdims=True)
    S = _spline_matrix(N_FRAMES, n_samples)
    delay_interp = S @ np.asarray(delay, np.float32)
    coeff_interp = S @ coeff
    z_l = np.floor(delay_interp).astype(np.int32)
    alfa = (delay_interp - z_l).astype(np.float32)
    b = coeff_interp
    v0 = -(1.0 - alfa) * b[:, 0]
    vmid = -(alfa[:, None] * b[:, : L_ORDER - 1]
             + (1.0 - alfa)[:, None] * b[:, 1:L_ORDER])
    vL = -alfa * b[:, -1]
    vals = np.concatenate([v0[:, None], vmid, vL[:, None]], 1).astype(np.float32)
    x = np.zeros(n_samples, np.float32)
    exc = np.asarray(exc, np.float32)
    x[: exc.shape[0]] = exc
    return vals, z_l, x


def _build_wts(vals, z_l, n_samples):
    """Dense per-chunk matmul weights, already transposed into lhsT layout.

    Returns (wts, basis) with wts [n_chunks, 5*128, 128] fp32 where:
      wts[c, 128g + p, m] = W[c][m, 128g + p]   (g = 0..3, window blocks)
      wts[c, 512 + p, m]  = L[c][m, p]          (p < 64, correction block)
    W[c][i, k] multiplies window sample y[128c - 512 + k] into output i;
    L[c][i, k] multiplies within-chunk y[128c + k] (k < 64) into output i.
    basis = max feedback lag (the needed width of the window basis)."""
    n_chunks = n_samples // CHUNK
    t = np.arange(n_samples)
    lag = 1 + z_l[:, None] + np.arange(6)[None, :]       # [T, 6]
    assert (lag[:, 0] >= CORR).all(), "delay too small for correction width"
    basis = int(lag.max())
    assert basis <= WIN - CORR, "delay too large for window"
    src = t[:, None] - lag                                # absolute read pos
    i_in_chunk = t % CHUNK
    k_win = WIN + i_in_chunk[:, None] - lag               # window col if < WIN
    wts = np.zeros((n_chunks, 5 * CHUNK, CHUNK), np.float32)
    c_of_t = t // CHUNK
    for j in range(6):
        valid = src[:, j] >= 0
        kw = k_win[:, j]
        in_window = valid & (kw < WIN)
        # window part: wts[c, kw, i] = vals[t, j]
        tw = t[in_window]
        wts[c_of_t[tw], kw[tw], i_in_chunk[tw]] += vals[tw, j]
        in_chunk = valid & (kw >= WIN)
        tc = t[in_chunk]
        kc = kw[tc] - WIN
        assert (kc < CORR).all()
        wts[c_of_t[tc], WIN + kc, i_in_chunk[tc]] += vals[tc, j]
    return wts, basis


def _fold_corr(wts_seg):
    """Fold each chunk's within-chunk correction into the weights of its
    in-segment readers, so the ring can store *uncorrected* columns and the
    correction matmul leaves the serial chunk-to-chunk dependency chain.

    Stored column of a corr-active chunk w: rows < CORR are true, rows >=
    CORR carry +psum2 = +(Lc @ y_lo).  A reader's true contribution is
    W @ true = W @ stored - W[:, CORR:] @ Lc[CORR:, :CORR] @ stored[:CORR],
    so fold:  lhsT_B[0:CORR] -= LcT[0:CORR, CORR:] @ lhsT_B[CORR:].
    Exact algebra; modifies wts_seg in place and returns it."""
    wts_seg = wts_seg.copy()
    n = wts_seg.shape[0]
    blocks = wts_seg.reshape(n, 5, CHUNK, CHUNK)
    corr_active = np.abs(blocks[:, 4]).reshape(n, -1).max(-1) > 0
    for w in range(n):
        if not corr_active[w]:
            continue
        corrT = blocks[w, 4]                      # [p, m] = Lc[m, p]
        for r in range(w + 1, min(w + 5, n)):
            g = w - r + 4
            blk = blocks[r, g]
            blk[0:CORR] -= corrT[0:CORR, CORR:] @ blk[CORR:]
    return wts_seg


def _union_plans(wts_segs, corr_pos_fn):
    """Shared (SPMD) per-position plans = union of active blocks across the
    per-core segments, plus per-phase correction positions.

    wts_segs: list of per-core [cps, 640, 128] arrays (already folded).
    corr_pos_fn(pos, corr_union) -> bool: whether position `pos` carries a
    correction matmul in the program.
    Returns (plans, packed_list): plans[c] = (wblocks, corr);
    packed_list[s] = [n_blocks, 128, 128] for core s (zero-padded where that
    core's block is inactive)."""
    cps = wts_segs[0].shape[0]
    act = np.stack([
        np.abs(w.reshape(cps, 5, -1)).max(-1) > 0 for w in wts_segs
    ])                                            # [n_seg, cps, 5]
    union = act.any(0)                            # [cps, 5]
    plans = []
    for c in range(cps):
        wblocks = [g for g in range(4) if union[c, g]]
        if not wblocks:
            wblocks = [3]
        plans.append((wblocks, bool(corr_pos_fn(c, union[c, 4]))))
    packed_list = []
    for w in wts_segs:
        blocks = w.reshape(cps, 5, CHUNK, CHUNK)
        out = []
        for c, (wblocks, corr) in enumerate(plans):
            sel = list(wblocks) + ([4] if corr else [])
            out.append(blocks[c, sel])
        packed_list.append(np.ascontiguousarray(np.concatenate(out, 0)))
    return plans, packed_list


# ----------------------------------------------------------------------------
# bass program builder
# ----------------------------------------------------------------------------

def _build_recur_nc(plans, n_blocks, nrhs, basis, want_y, want_t,
                    fast_mm=False, want_h=False):
    key = ("recur", tuple((tuple(wb), co) for wb, co in plans), n_blocks,
           nrhs, basis, want_y, want_t, fast_mm, want_h)
    if key in _NC_CACHE:
        return _NC_CACHE[key]
    nc = _build_recur_nc_impl(plans, n_blocks, nrhs, basis, want_y, want_t,
                              fast_mm, want_h)
    _NC_CACHE[key] = nc
    return nc


def _build_recur_nc_impl(plans, n_blocks, nrhs, basis, want_y, want_t,
                         fast_mm, want_h):
    """Bass/Tile program running the chunked recurrence with `nrhs`
    right-hand-side columns; per-chunk blocks given by `plans`.

    The ring stores *uncorrected* columns (corrections are folded into the
    reader weights on the host — see _fold_corr), so the chunk-to-chunk
    serial chain is just matmul -> subtract.  Where plans[c] includes the
    correction block, a correction matmul computes the true values off the
    chain for the outputs (yout / final window).

    Inputs:  wts   [n_blocks, 128, 128] f32  (packed lhsT blocks)
             xin   [128, n_chunks]      f32  (x, chunk-column layout)
             ring0 [128, 4, nrhs]       f32  (initial window columns, true)
    Outputs: tout  [128, 4, nrhs]       f32  (true final window, if want_t)
             yout  [128, n_chunks]      f32  (true outputs, if want_y)
    """
    n_chunks = len(plans)
    nb_max = max(len(wb) + int(co) for wb, co in plans)
    # float32r: PE reads fp32 bits but multiplies at FP22 in a single pass
    # (vs 2 half-speed passes for true fp32) — ~4x faster at wide N.
    MMDT = mybir.dt.float32r if fast_mm else F32
    nc = bacc.Bacc("TRN2", target_bir_lowering=False, debug=False,
                   num_devices=N_CORES, enable_partition_id=False)
    wts = nc.dram_tensor("wts", [n_blocks, CHUNK, CHUNK], MMDT,
                         kind="ExternalInput")
    xin = nc.dram_tensor("xin", [CHUNK, n_chunks], F32, kind="ExternalInput")
    ring0 = nc.dram_tensor("ring0", [CHUNK, 4, nrhs], MMDT,
                           kind="ExternalInput")
    tout = yout = hout = None
    if want_t:
        tout = nc.dram_tensor("tout", [CHUNK, 4, nrhs], F32,
                              kind="ExternalOutput")
    if want_y:
        yout = nc.dram_tensor("yout", [CHUNK, n_chunks], F32,
                              kind="ExternalOutput")
    if want_h:
        # uncorrected response operators (= ring columns), 4 chunks a batch
        hout = nc.dram_tensor("hout", [n_chunks // 4, CHUNK, 4, nrhs], MMDT,
                              kind="ExternalOutput")

    with tile.TileContext(nc) as tc:
        with (
            tc.tile_pool(name="state", bufs=1) as state,
            tc.tile_pool(name="wpool", bufs=8) as wpool,
            tc.tile_pool(name="psum", bufs=4, space="PSUM") as ppool,
        ):
            ring = state.tile([CHUNK, RING, nrhs], MMDT)
            xin_sb = state.tile([CHUNK, n_chunks], F32)
            # ring0 first: the first chunk's matmuls need it, xin can wait
            nc.sync.dma_start(ring[:, 4:8, :], ring0[:])
            nc.sync.dma_start(xin_sb[:], xin[:])
            yout_sb = trueout = xext = None
            if want_y:
                yout_sb = state.tile([CHUNK, n_chunks], F32)
            if want_t:
                trueout = state.tile([CHUNK, 4, nrhs], F32)
            if nrhs > 1:
                # x-extended rhs template: zeros except the particular col
                xext = state.tile([CHUNK, nrhs], F32)
                nc.vector.memset(xext[:], 0.0)

            lo = slice(0, CORR)
            hi = slice(CORR, CHUNK)
            off = 0
            for c in range(n_chunks):
                wblocks, corr = plans[c]
                nb = len(wblocks) + int(corr)
                wtile = wpool.tile([CHUNK, nb_max, CHUNK], MMDT, tag="wt")
                nc.sync.dma_start(
                    wtile[:, 0:nb, :],
                    wts[off: off + nb].rearrange("b p m -> p b m"),
                )
                off += nb
                psum = ppool.tile([CHUNK, nrhs], F32, tag="acc")
                for i, g in enumerate(wblocks):
                    col = (c + 4 + g) % RING
                    nc.tensor.matmul(
                        psum[:],
                        wtile[:, i, :],
                        ring[:, col, :],
                        start=(i == 0),
                        stop=(i == len(wblocks) - 1),
                    )
                rc = c % RING
                # stored (uncorrected) column — the serial chain tail
                if nrhs == 1:
                    nc.vector.tensor_sub(
                        ring[:, rc, :], xin_sb[:, c: c + 1], psum[:]
                    )
                else:
                    # refresh the particular column of the template (off the
                    # chain), then one fused op: ring_col = -psum + xext
                    nc.vector.tensor_copy(
                        xext[:, basis: basis + 1], xin_sb[:, c: c + 1])
                    nc.vector.scalar_tensor_tensor(
                        out=ring[:, rc, :], in0=psum[:], scalar=-1.0,
                        in1=xext[:], op0=mybir.AluOpType.mult,
                        op1=mybir.AluOpType.add,
                    )
                if want_h and c % 4 == 3:
                    base = (c - 3) % RING          # 0 or 4: contiguous 4 cols
                    nc.sync.dma_start(
                        hout[c // 4], ring[:, base: base + 4, :])
                # corrected outputs, off the chain
                psum2 = None
                if corr:
                    psum2 = ppool.tile([CHUNK, nrhs], F32, tag="corr")
                    nc.tensor.matmul(
                        psum2[:],
                        wtile[0:CORR, nb - 1, :],
                        ring[lo, rc, :],
                        start=True,
                        stop=True,
                    )
                if want_y:
                    if corr:
                        nc.vector.tensor_copy(
                            yout_sb[lo, c: c + 1], ring[lo, rc, :])
                        nc.vector.tensor_sub(
                            yout_sb[hi, c: c + 1], ring[hi, rc, :],
                            psum2[hi, :])
                    else:
                        nc.vector.tensor_copy(
                            yout_sb[:, c: c + 1], ring[:, rc, :])
                if want_t and c >= n_chunks - 4:
                    k = c - (n_chunks - 4)
                    if corr:
                        nc.vector.tensor_copy(
                            trueout[lo, k, :], ring[lo, rc, :])
                        nc.vector.tensor_sub(
                            trueout[hi, k, :], ring[hi, rc, :], psum2[hi, :])
                    else:
                        nc.vector.tensor_copy(trueout[:, k, :], ring[:, rc, :])

            assert n_chunks % RING == 0
            if want_t:
                nc.sync.dma_start(tout[:], trueout[:])
            if want_y:
                nc.sync.dma_start(yout[:], yout_sb[:])
    nc.compile()
    return nc


def _build_apply_nc(corr_flags, nrhs, fast_h=False):
    key = ("apply", tuple(corr_flags), nrhs)
    if key in _NC_CACHE:
        return _NC_CACHE[key]
    nc = _build_apply_nc_impl(corr_flags, nrhs)
    _NC_CACHE[key] = nc
    return nc


def _build_apply_nc_impl(corr_flags, nrhs):
    """Bass/Tile program applying the segment's true initial window to the
    stored per-chunk response operators from phase B:
        y[:, c] = H_c @ wvec      (fused multiply+reduce on the vector engine)
    then fixing the within-chunk correction for corr-active chunks.

    Inputs:  hseg  [n_chunks, 128, nrhs]  (phase B's hout)
             wb    [128, nrhs]            (wvec broadcast across partitions)
             cwts  [n_corr, 128, 128]     (correction lhsT blocks, packed)
    Outputs: yout  [128, n_chunks]
    """
    n_chunks = len(corr_flags)
    n_corr = int(np.sum(corr_flags))
    assert n_chunks % 4 == 0
    # H bytes are fp32 either way (float32r is just an fp32 PE read mode)
    HDT = F32
    nc = bacc.Bacc("TRN2", target_bir_lowering=False, debug=False,
                   num_devices=N_CORES, enable_partition_id=False)
    hseg = nc.dram_tensor("hseg", [n_chunks // 4, CHUNK, 4, nrhs], HDT,
                          kind="ExternalInput")
    wb = nc.dram_tensor("wb", [CHUNK, nrhs], F32, kind="ExternalInput")
    cwts = nc.dram_tensor("cwts", [max(n_corr, 1), CHUNK, CHUNK], F32,
                          kind="ExternalInput")
    yout = nc.dram_tensor("yout", [CHUNK, n_chunks], F32,
                          kind="ExternalOutput")

    with tile.TileContext(nc) as tc:
        with (
            tc.tile_pool(name="state", bufs=1) as state,
            tc.tile_pool(name="hpool", bufs=8) as hpool,
            tc.tile_pool(name="spool", bufs=4) as spool,
            tc.tile_pool(name="psum", bufs=4, space="PSUM") as ppool,
        ):
            wb_sb = state.tile([CHUNK, nrhs], F32)
            nc.sync.dma_start(wb_sb[:], wb[:])
            # all correction blocks stay resident in SBUF
            call_sb = state.tile([CHUNK, max(n_corr, 1), CHUNK], F32)
            nc.sync.dma_start(
                call_sb[:], cwts[:].rearrange("b p m -> p b m"))
            yout_sb = state.tile([CHUNK, n_chunks], F32)
            # pass 1: all the multiply+reduce work, back-to-back on DVE
            # (keeping the correction fix-ups out of the in-order DVE stream
            # here avoids head-of-line blocking behind the PE matmuls)
            for c0 in range(0, n_chunks, 4):
                htile = hpool.tile([CHUNK, 4, nrhs], HDT, tag="h")
                nc.sync.dma_start(htile[:], hseg[c0 // 4])
                for c in range(c0, c0 + 4):
                    # the elementwise product is discarded (only the fp32
                    # accumulator is used); bf16 out may enable a faster
                    # DVE mode without touching accumulation precision
                    scratch = spool.tile([CHUNK, nrhs], mybir.dt.bfloat16,
                                         tag="s")
                    nc.vector.scalar_tensor_tensor(
                        out=scratch[:], in0=htile[:, c - c0, :], scalar=1.0,
                        in1=wb_sb[:], op0=mybir.AluOpType.mult,
                        op1=mybir.AluOpType.mult,
                        accum_out=yout_sb[:, c: c + 1],
                    )
            # pass 2: correction fix-ups
            ci = 0
            for c in range(n_chunks):
                if not corr_flags[c]:
                    continue
                psum2 = ppool.tile([CHUNK, 1], F32, tag="corr")
                nc.tensor.matmul(
                    psum2[:], call_sb[0:CORR, ci, :],
                    yout_sb[0:CORR, c: c + 1],
                    start=True, stop=True,
                )
                ci += 1
                nc.vector.tensor_sub(
                    yout_sb[CORR:CHUNK, c: c + 1],
                    yout_sb[CORR:CHUNK, c: c + 1],
                    psum2[CORR:CHUNK, :],
                )
            assert ci == n_corr
            nc.sync.dma_start(yout[:], yout_sb[:])
    nc.compile()
    return nc


# ----------------------------------------------------------------------------
# host orchestration
# ----------------------------------------------------------------------------

def _run(nc, in_maps, tag):
    trace = bool(int(os.environ.get("DIFFKS_TRACE", "0")))
    kw = {}
    tcs = os.environ.get("DIFFKS_TRACE_CORES", "")
    if trace and tcs:
        kw["trace_cores"] = [int(x) for x in tcs.split(",")]
    res = run_bass_kernel_spmd(
        nc, in_maps, core_ids=list(range(len(in_maps))), trace=trace, **kw
    )
    LAST_RESULTS[tag] = res
    return res.results


def _basis_ring0(basis):
    """Initial window columns for phase B: basis b is a unit vector at
    window position (WIN-basis)+b; the particular column starts at zero."""
    nrhs = basis + 1
    r0 = np.zeros((CHUNK, 4, nrhs), np.float32)
    for b in range(basis):
        p = (WIN - basis) + b
        r0[p % CHUNK, p // CHUNK, b] = 1.0
    return r0


def kernel(delay_len_frames, raw_coeff_frames, excitation, n_samples):
    n = int(n_samples)
    assert n == N_SAMPLES, f"kernel hardcoded for {N_SAMPLES}, got {n}"
    LAST_RESULTS.clear()

    vals, z_l, x = _preprocess(delay_len_frames, raw_coeff_frames,
                               excitation, n)
    wts, basis = _build_wts(vals, z_l, n)
    nrhs = basis + 1
    n_chunks = n // CHUNK
    xin = np.ascontiguousarray(x.reshape(n_chunks, CHUNK).T)   # [128, n_chunks]

    mode = os.environ.get("DIFFKS_MODE", "seg")
    if mode == "seq":
        # single-chain: every core runs the full sequence; take core 0
        folded = _fold_corr(wts)
        plans, packed = _union_plans([folded], lambda c, cu: cu)
        nc = _build_recur_nc(plans, packed[0].shape[0], 1, basis,
                             want_y=True, want_t=False)
        in_map = {
            "wts": packed[0],
            "xin": xin,
            "ring0": np.zeros((CHUNK, 4, 1), np.float32),
        }
        outs = _run(nc, [in_map] * N_CORES, "seq")
        y = outs[0]["yout"].T.reshape(n)                        # [128, nc] -> t
        return y.astype(np.float32)

    # ---- segmented: 8 cores, phase B -> host combine -> phase C ----
    cps = n_chunks // N_CORES                                   # chunks/segment
    seg_wts = [_fold_corr(wts[s * cps: (s + 1) * cps])
               for s in range(N_CORES)]
    seg_xin = [np.ascontiguousarray(xin[:, s * cps: (s + 1) * cps])
               for s in range(N_CORES)]

    # phase B: basis + particular responses; correction only needed for the
    # final window columns (last 4 chunk positions)
    fast_b = bool(int(os.environ.get("DIFFKS_FASTB", "0")))
    use_apply = bool(int(os.environ.get("DIFFKS_APPLY", "1")))
    plansB, packedB = _union_plans(
        seg_wts, lambda c, cu: cu and c >= cps - 4)
    ncB = _build_recur_nc(plansB, packedB[0].shape[0], nrhs, basis,
                          want_y=False, want_t=True, fast_mm=fast_b,
                          want_h=use_apply)
    r0 = _basis_ring0(basis)
    in_maps = [
        {"wts": packedB[s], "xin": seg_xin[s], "ring0": r0}
        for s in range(N_CORES)
    ]
    outsB = _run(ncB, in_maps, "phaseB")

    # host combine: chain transfer operators (8 tiny matvecs)
    wins = [np.zeros(WIN, np.float32)]
    for s in range(N_CORES):
        T = outsB[s]["tout"]            # [128, 4, nrhs]
        T = T.transpose(1, 0, 2).reshape(WIN, nrhs)   # window pos major
        w_next = T[:, :basis] @ wins[s][WIN - basis:] + T[:, basis]
        wins.append(w_next.astype(np.float32))

    if use_apply:
        # apply pass: y[:, c] = H_c @ [w; 1], plus correction fix-ups
        corr_flags = [
            bool(np.any([
                np.abs(w.reshape(cps, 5, CHUNK, CHUNK)[c, 4]).max() > 0
                for w in seg_wts
            ]))
            for c in range(cps)
        ]
        ncA = _build_apply_nc(corr_flags, nrhs, fast_h=fast_b)
        n_corr = max(int(np.sum(corr_flags)), 1)
        in_maps = []
        for s in range(N_CORES):
            blocks = seg_wts[s].reshape(cps, 5, CHUNK, CHUNK)
            cw = np.zeros((n_corr, CHUNK, CHUNK), np.float32)
            ci = 0
            for c in range(cps):
                if corr_flags[c]:
                    cw[ci] = blocks[c, 4]
                    ci += 1
            wv = np.concatenate(
                [wins[s][WIN - basis:], np.ones(1, np.float32)])
            wb = np.ascontiguousarray(
                np.broadcast_to(wv, (CHUNK, nrhs))).astype(np.float32)
            in_maps.append({
                "hseg": outsB[s]["hout"],
                "wb": wb,
                "cwts": cw,
            })
        outsC = _run(ncA, in_maps, "apply")
    else:
        # phase C: re-run with the true initial windows
        plansC, packedC = _union_plans(seg_wts, lambda c, cu: cu)
        ncC = _build_recur_nc(plansC, packedC[0].shape[0], 1, basis,
                              want_y=True, want_t=False)
        in_maps = [
            {
                "wts": packedC[s],
                "xin": seg_xin[s],
                "ring0": np.ascontiguousarray(
                    wins[s].reshape(4, CHUNK).T.reshape(CHUNK, 4, 1)
                ),
            }
            for s in range(N_CORES)
        ]
        outsC = _run(ncC, in_maps, "phaseC")

    y = np.concatenate(
        [outsC[s]["yout"].T.reshape(cps * CHUNK) for s in range(N_CORES)]
    )
    return y.astype(np.float32)



# revision 4
# speedup vs baseline: 1.5720x; 1.5720x over previous
"""Trainium2 Bass kernel for nn_DiffKS (differentiable Karplus-Strong).

Computation: y[t] = x[t] - sum_{j=0..5} vals[t,j] * y[t - 1 - z_l[t] - j],
vals / z_l from spline-interpolated delay & coefficient trajectories.
The feedback lag is always >= ~93, so 128-sample chunks are computed as
dense banded matmuls against a 512-sample window of past output (ring
columns in SBUF) plus a folded within-chunk correction (_fold_corr).

v2 strategy (32 segments, two SPMD programs, host combine between):
  - the 65536 samples are 512 chunks = 32 segments x 16 chunks; segments
    are grouped into 4 "slots" of 8 (one segment per core per slot),
    sorted by per-segment basis width so each slot's recurrence runs with
    the narrowest possible RHS count (nrhs = max initial-window reach of
    its segments + 1 particular column).
  - phase B (parallel): every core runs its 4 segments as 4 independent
    interleaved chunk-chains (hides the serial matmul->subtract chain and
    keeps the PE at max p-state) with nrhs_j columns: identity window
    basis + the excitation particular column.  Everything fp16 (weights,
    ring state, final-window transfer outputs); PE accumulates f32.
    Weights live SBUF-resident, prefetched with a few large
    partition-major DMAs.
  - combine (host, tiny, f64): chain the 32 transfer operators to get
    every segment's true initial window.
  - phase C (parallel): same chained recurrence with nrhs=1 and the true
    initial windows, emitting the corrected outputs.
fp16 end-to-end rel err ~5e-4 (validated in simulation + hardware).
"""

import os
import numpy as np

import concourse.bacc as bacc
import concourse.tile as tile
import concourse.mybir as mybir
from concourse.bass_utils import run_bass_kernel_spmd


def _ensure_ntff_hook():
    """The agent image's `antenv` stub lacks `axon_hooks`, which
    `run_bass_kernel_spmd(trace=True)` needs under axon for NTFF capture.
    Recreate the ctypes-based hook `trn_agent_boot.trn_boot` would install
    on images where the module exists."""
    try:
        from antenv.axon_hooks import get_axon_ntff_profile_hook  # noqa: F401
        return
    except ImportError:
        pass
    import contextlib
    import ctypes
    import sys
    import types

    so_path = "/opt/axon/libaxon_pjrt.so"
    if not os.path.exists(so_path):
        return
    lib = ctypes.CDLL(so_path)
    if not hasattr(lib, "axon_start_nrt_profile"):
        return
    lib.axon_start_nrt_profile.argtypes = [
        ctypes.POINTER(ctypes.c_int64), ctypes.c_size_t]
    lib.axon_start_nrt_profile.restype = ctypes.c_int64
    lib.axon_stop_nrt_profile.argtypes = [ctypes.c_char_p]
    lib.axon_stop_nrt_profile.restype = ctypes.c_int64

    @contextlib.contextmanager
    def _hook(output_dir, device_ids):
        import jax
        jax.devices()
        if device_ids:
            ids = (ctypes.c_int64 * len(device_ids))(*device_ids)
            rc = lib.axon_start_nrt_profile(ids, len(device_ids))
        else:
            rc = lib.axon_start_nrt_profile(None, 0)
        if rc != 0:
            raise RuntimeError(f"axon_start_nrt_profile rc={rc}")
        try:
            yield
        finally:
            n = lib.axon_stop_nrt_profile(str(output_dir).encode())
            if n <= 0:
                print(f"ntff profile: {n} file(s) written to {output_dir}",
                      file=sys.stderr)

    mod = types.ModuleType("antenv.axon_hooks")
    mod._hook = _hook
    mod.get_axon_ntff_profile_hook = lambda: _hook
    mod.set_axon_ntff_profile_hook = lambda h: setattr(mod, "_hook", h)
    import antenv
    antenv.axon_hooks = mod
    sys.modules["antenv.axon_hooks"] = mod


_ensure_ntff_hook()

F32 = mybir.dt.float32
F16 = mybir.dt.float16

N_SAMPLES = 65536
N_FRAMES = 64
L_ORDER = 5
CHUNK = 128
WIN = 512            # window length the chunk matmuls see (4 ring cols)
RING = 8             # ring columns in SBUF (power of two, >= 5)
CORR = 64            # within-chunk correction width (needs z_l >= 63)
N_CORES = 8
N_SLOTS = 4
CPS = 16             # chunks per segment
N_SEG = N_SLOTS * N_CORES
W_DMA_BLOCKS = 28    # weight blocks per prefetch DMA

# filled by kernel() with per-phase profiling results for the test harness
LAST_RESULTS = {}

_NC_CACHE = {}


# ----------------------------------------------------------------------------
# host-side preprocessing
# ----------------------------------------------------------------------------

_SPLINE_CACHE = {}


def _spline_matrix(n_in, n_out):
    """Static [n_out, n_in] natural-cubic-spline interpolation matrix for
    uniform knots (input-independent)."""
    key = (n_in, n_out)
    if key in _SPLINE_CACHE:
        return _SPLINE_CACHE[key]
    t_in = np.linspace(0.0, 1.0, n_in)
    t_out = np.linspace(0.0, 1.0, n_out)
    n = n_in
    h = t_in[1:] - t_in[:-1]
    R = np.zeros((n - 2, n))
    for i in range(n - 2):
        R[i, i] += 6.0 / h[i]
        R[i, i + 1] += -6.0 / h[i] - 6.0 / h[i + 1]
        R[i, i + 2] += 6.0 / h[i + 1]
    A = (
        np.diag(2.0 * (h[:-1] + h[1:]))
        + np.diag(h[1:-1], 1)
        + np.diag(h[1:-1], -1)
    )
    M = np.zeros((n, n))
    M[1:-1] = np.linalg.solve(A, R)
    idx = np.clip(np.searchsorted(t_in, t_out, side="right") - 1, 0, n - 2)
    dt = t_out - t_in[idx]
    S = np.zeros((n_out, n))
    eye = np.eye(n)
    for r in range(n_out):
        i = idx[r]
        b = (eye[i + 1] - eye[i]) / h[i] - h[i] * (2.0 * M[i] + M[i + 1]) / 6.0
        c = M[i] / 2.0
        d = (M[i + 1] - M[i]) / (6.0 * h[i])
        S[r] = eye[i] + b * dt[r] + c * dt[r] ** 2 + d * dt[r] ** 3
    S = S.astype(np.float32)
    _SPLINE_CACHE[key] = S
    return S


def _preprocess(delay, raw, exc, n_samples):
    sig = 1.0 / (1.0 + np.exp(-np.asarray(raw, np.float32)))
    coeff = sig / sig.sum(-1, keepdims=True)
    S = _spline_matrix(N_FRAMES, n_samples)
    delay_interp = S @ np.asarray(delay, np.float32)
    coeff_interp = S @ coeff
    z_l = np.floor(delay_interp).astype(np.int32)
    alfa = (delay_interp - z_l).astype(np.float32)
    b = coeff_interp
    v0 = -(1.0 - alfa) * b[:, 0]
    vmid = -(alfa[:, None] * b[:, : L_ORDER - 1]
             + (1.0 - alfa)[:, None] * b[:, 1:L_ORDER])
    vL = -alfa * b[:, -1]
    vals = np.concatenate([v0[:, None], vmid, vL[:, None]], 1).astype(np.float32)
    x = np.zeros(n_samples, np.float32)
    exc = np.asarray(exc, np.float32)
    x[: exc.shape[0]] = exc
    return vals, z_l, x


def _build_wts(vals, z_l, n_samples):
    """Dense per-chunk matmul weights in lhsT layout (see v1 docstring).
    wts [n_chunks, 5*128, 128]: groups 0..3 = window blocks, 4 = within-
    chunk correction block."""
    n_chunks = n_samples // CHUNK
    t = np.arange(n_samples)
    lag = 1 + z_l[:, None] + np.arange(6)[None, :]
    assert (lag[:, 0] >= CORR).all()
    basis = int(lag.max())
    assert basis <= WIN - CORR
    src = t[:, None] - lag
    i_in_chunk = t % CHUNK
    k_win = WIN + i_in_chunk[:, None] - lag
    wts = np.zeros((n_chunks, 5 * CHUNK, CHUNK), np.float32)
    c_of_t = t // CHUNK
    for j in range(6):
        valid = src[:, j] >= 0
        kw = k_win[:, j]
        in_window = valid & (kw < WIN)
        tw = t[in_window]
        wts[c_of_t[tw], kw[tw], i_in_chunk[tw]] += vals[tw, j]
        in_chunk = valid & (kw >= WIN)
        tc = t[in_chunk]
        kc = kw[tc] - WIN
        assert (kc < CORR).all()
        wts[c_of_t[tc], WIN + kc, i_in_chunk[tc]] += vals[tc, j]
    return wts, basis


def _fold_corr(wts_seg):
    """Fold each chunk's within-chunk correction into its in-segment
    readers so ring columns can stay uncorrected (exact algebra)."""
    wts_seg = wts_seg.copy()
    n = wts_seg.shape[0]
    blocks = wts_seg.reshape(n, 5, CHUNK, CHUNK)
    corr_active = np.abs(blocks[:, 4]).reshape(n, -1).max(-1) > 0
    for w in range(n):
        if not corr_active[w]:
            continue
        corrT = blocks[w, 4]
        for r in range(w + 1, min(w + 5, n)):
            g = w - r + 4
            blk = blocks[r, g]
            blk[0:CORR] -= corrT[0:CORR, CORR:] @ blk[CORR:]
    return wts_seg


def _segment_layout(vals, z_l, wts):
    """Slot assignment + per-slot plans.

    Returns dict with:
      slot_segs [N_SLOTS][N_CORES] -> segment index (time order 0..31)
      nrhs      [N_SLOTS]          -> basis_j + 1
      plans_b / plans_c: [N_SLOTS][CPS] -> (window_gs, corr_flag)
      seg_fold  [N_SEG] -> folded weights [CPS, 5*128, 128] f32
    """
    lag = 1 + z_l[:, None] + np.arange(6)[None, :]
    t = np.arange(N_SAMPLES)
    seglen = CPS * CHUNK
    bas = []
    for s in range(N_SEG):
        t0 = s * seglen
        reach = lag[t0:t0 + seglen] - (t[t0:t0 + seglen] - t0)[:, None]
        bas.append(int(reach.max()))
    bas = np.array(bas)
    order = np.argsort(bas, kind="stable")
    slot_segs = [order[8 * j: 8 * j + 8].tolist() for j in range(N_SLOTS)]
    nrhs = [int(bas[g].max()) + 1 for g in slot_segs]

    seg_fold = [_fold_corr(wts[s * CPS:(s + 1) * CPS]) for s in range(N_SEG)]
    act = np.stack([
        np.abs(f.reshape(CPS, 5, -1)).max(-1) > 0 for f in seg_fold
    ])                                            # [N_SEG, CPS, 5]
    plans_b, plans_c = [], []
    for j in range(N_SLOTS):
        u = act[slot_segs[j]].any(0)              # [CPS, 5]
        pb, pcn = [], []
        for c in range(CPS):
            wb = [g for g in range(4) if u[c, g]]
            if not wb:
                wb = [3]
            pb.append((wb, bool(u[c, 4] and c >= CPS - 4)))
            pcn.append((wb, bool(u[c, 4])))
        plans_b.append(pb)
        plans_c.append(pcn)
    return dict(slot_segs=slot_segs, nrhs=nrhs, basis=[n - 1 for n in nrhs],
                plans_b=plans_b, plans_c=plans_c, seg_fold=seg_fold, bas=bas)


def _pack_wts(layout, plans, core):
    """Per-core packed fp16 weights, partition-major [128, NB, 128],
    in emission order: for c in 0..CPS-1: for j in 0..N_SLOTS-1:
    window blocks then (if flagged) the correction block."""
    out = []
    for c in range(CPS):
        for j in range(N_SLOTS):
            seg = layout["slot_segs"][j][core]
            blocks = layout["seg_fold"][seg].reshape(CPS, 5, CHUNK, CHUNK)
            wb, co = plans[j][c]
            sel = list(wb) + ([4] if co else [])
            out.append(blocks[c, sel])
    packed = np.concatenate(out, 0)               # [NB, 128, 128]
    return np.ascontiguousarray(
        packed.transpose(1, 0, 2)).astype(np.float16)  # [128, NB, 128]


def _plan_nblocks(plans):
    return sum(len(wb) + int(co)
               for j in range(N_SLOTS) for wb, co in plans[j])


# ----------------------------------------------------------------------------
# bass program builder
# ----------------------------------------------------------------------------

def _plan_key(plans):
    return tuple(tuple((tuple(wb), co) for wb, co in p) for p in plans)


def _build_recur_nc(plans, nrhs_list, want_t, want_y):
    key = ("recur2", _plan_key(plans), tuple(nrhs_list), want_t, want_y)
    if key in _NC_CACHE:
        return _NC_CACHE[key]
    nc = _build_recur_nc_impl(plans, nrhs_list, want_t, want_y)
    _NC_CACHE[key] = nc
    return nc


def _build_recur_nc_impl(plans, nrhs_list, want_t, want_y):
    """Chained chunk recurrence over N_SLOTS independent interleaved
    chains (one segment per slot per core), nrhs_list[j] RHS columns.

    Inputs:  wts   [128, NB, 128] f16   (packed lhsT blocks, emission order)
             xin   [128, CPS * N_SLOTS] f32 (position p = c*N_SLOTS+j)
             ring0_j [128, 4, nrhs_j] f16 per slot (initial window columns)
    Outputs: tout_j [128, 4, nrhs_j] f16 per slot (true final window) if want_t
             yout  [128, CPS * N_SLOTS] f32 (corrected outputs) if want_y
    """
    NB = _plan_nblocks(plans)
    nc = bacc.Bacc("TRN2", target_bir_lowering=False, debug=False,
                   num_devices=N_CORES, enable_partition_id=False)
    wts = nc.dram_tensor("wts", [CHUNK, NB, CHUNK], F16, kind="ExternalInput")
    xin = nc.dram_tensor("xin", [CHUNK, CPS * N_SLOTS], F32,
                         kind="ExternalInput")
    ring0 = [nc.dram_tensor(f"ring0_{j}", [CHUNK, 4, nrhs_list[j]], F16,
                            kind="ExternalInput") for j in range(N_SLOTS)]
    tout = yout = None
    if want_t:
        tout = [nc.dram_tensor(f"tout_{j}", [CHUNK, 4, nrhs_list[j]], F16,
                               kind="ExternalOutput") for j in range(N_SLOTS)]
    if want_y:
        yout = nc.dram_tensor("yout", [CHUNK, CPS * N_SLOTS], F32,
                              kind="ExternalOutput")

    with tile.TileContext(nc) as tc:
        with (
            tc.tile_pool(name="state", bufs=1) as state,
            tc.tile_pool(name="psum", bufs=1, space="PSUM") as ppool,
        ):
            wsb = state.tile([CHUNK, NB, CHUNK], F16)
            rings = [state.tile([CHUNK, RING, nrhs_list[j]], F16,
                                name=f"ring{j}")
                     for j in range(N_SLOTS)]
            xin_sb = state.tile([CHUNK, CPS * N_SLOTS], F32)
            # initial window columns + x first (first chunks need them),
            # then the weights in emission-order runs so early matmuls
            # unblock as soon as their covering DMA lands
            for j in range(N_SLOTS):
                nc.sync.dma_start(rings[j][:, 4:8, :], ring0[j][:])
            nc.sync.dma_start(xin_sb[:], xin[:])
            for a in range(0, NB, W_DMA_BLOCKS):
                b = min(a + W_DMA_BLOCKS, NB)
                nc.sync.dma_start(wsb[:, a:b, :], wts[:, a:b, :])

            trueout = None
            if want_t:
                trueout = [state.tile([CHUNK, 4, nrhs_list[j]], F16,
                                      name=f"trueout{j}")
                           for j in range(N_SLOTS)]
            yout_sb = None
            if want_y:
                yout_sb = state.tile([CHUNK, CPS * N_SLOTS], F32)
            # x-extended rhs templates: zeros except the particular column
            xext = [state.tile([CHUNK, nrhs_list[j]], F32,
                               name=f"xext{j}")
                    for j in range(N_SLOTS)]
            for j in range(N_SLOTS):
                nc.vector.memset(xext[j][:], 0.0)

            lo = slice(0, CORR)
            hi = slice(CORR, CHUNK)
            off = 0
            for c in range(CPS):
                for j in range(N_SLOTS):
                    nrhs = nrhs_list[j]
                    basis = nrhs - 1
                    ring = rings[j]
                    wb, corr = plans[j][c]
                    pos = c * N_SLOTS + j
                    psum = ppool.tile([CHUNK, nrhs], F32, tag=f"acc{j}")
                    for i, g in enumerate(wb):
                        col = (c + 4 + g) % RING
                        nc.tensor.matmul(
                            psum[:],
                            wsb[:, off + i, :],
                            ring[:, col, :],
                            start=(i == 0),
                            stop=(i == len(wb) - 1),
                        )
                    rc = c % RING
                    # refresh particular column of the template, then one
                    # fused op: ring_col = -psum + xext  (stored fp16)
                    nc.vector.tensor_copy(
                        xext[j][:, basis: basis + 1], xin_sb[:, pos: pos + 1])
                    nc.vector.scalar_tensor_tensor(
                        out=ring[:, rc, :], in0=psum[:], scalar=-1.0,
                        in1=xext[j][:], op0=mybir.AluOpType.mult,
                        op1=mybir.AluOpType.add,
                    )
                    # corrected outputs, off the chain
                    psum2 = None
                    if corr:
                        psum2 = ppool.tile([CHUNK, nrhs], F32, tag=f"corr{j}")
                        nc.tensor.matmul(
                            psum2[:],
                            wsb[lo, off + len(wb), :],
                            ring[lo, rc, :],
                            start=True,
                            stop=True,
                        )
                    off += len(wb) + int(corr)
                    if want_y:
                        if corr:
                            nc.vector.tensor_copy(
                                yout_sb[lo, pos: pos + 1], ring[lo, rc, :])
                            nc.vector.tensor_sub(
                                yout_sb[hi, pos: pos + 1], ring[hi, rc, :],
                                psum2[hi, :])
                        else:
                            nc.vector.tensor_copy(
                                yout_sb[:, pos: pos + 1], ring[:, rc, :])
                    if want_t and c >= CPS - 4:
                        k = c - (CPS - 4)
                        if corr:
                            nc.vector.tensor_copy(
                                trueout[j][lo, k, :], ring[lo, rc, :])
                            nc.vector.tensor_sub(
                                trueout[j][hi, k, :], ring[hi, rc, :],
                                psum2[hi, :])
                        else:
                            nc.vector.tensor_copy(
                                trueout[j][:, k, :], ring[:, rc, :])
            assert off == NB

            if want_t:
                for j in range(N_SLOTS):
                    nc.sync.dma_start(tout[j][:], trueout[j][:])
            if want_y:
                nc.sync.dma_start(yout[:], yout_sb[:])
    nc.compile()
    return nc


# ----------------------------------------------------------------------------
# host orchestration
# ----------------------------------------------------------------------------

def _run(nc, in_maps, tag):
    trace = bool(int(os.environ.get("DIFFKS_TRACE", "0")))
    kw = {}
    tcs = os.environ.get("DIFFKS_TRACE_CORES", "")
    if trace and tcs:
        kw["trace_cores"] = [int(x) for x in tcs.split(",")]
    res = run_bass_kernel_spmd(
        nc, in_maps, core_ids=list(range(len(in_maps))), trace=trace, **kw
    )
    LAST_RESULTS[tag] = res
    return res.results


def _basis_ring0(basis):
    """Initial window columns for phase B: basis b is a unit vector at
    window position (WIN-basis)+b; the particular column starts at zero."""
    nrhs = basis + 1
    r0 = np.zeros((CHUNK, 4, nrhs), np.float16)
    for b in range(basis):
        p = (WIN - basis) + b
        r0[p % CHUNK, p // CHUNK, b] = 1.0
    return r0


def kernel(delay_len_frames, raw_coeff_frames, excitation, n_samples):
    n = int(n_samples)
    assert n == N_SAMPLES, f"kernel hardcoded for {N_SAMPLES}, got {n}"
    LAST_RESULTS.clear()

    vals, z_l, x = _preprocess(delay_len_frames, raw_coeff_frames,
                               excitation, n)
    wts, _ = _build_wts(vals, z_l, n)
    layout = _segment_layout(vals, z_l, wts)
    slot_segs = layout["slot_segs"]
    nrhs_list = layout["nrhs"]
    n_chunks = n // CHUNK
    xin_all = x.reshape(n_chunks, CHUNK).T          # [128, n_chunks]

    def core_xin(core):
        xi = np.zeros((CHUNK, CPS * N_SLOTS), np.float32)
        for j in range(N_SLOTS):
            seg = slot_segs[j][core]
            for c in range(CPS):
                xi[:, c * N_SLOTS + j] = xin_all[:, seg * CPS + c]
        return xi

    # ---- phase B ----
    ncB = _build_recur_nc(layout["plans_b"], nrhs_list,
                          want_t=True, want_y=False)
    in_maps = []
    r0s = [_basis_ring0(nrhs_list[j] - 1) for j in range(N_SLOTS)]
    for s in range(N_CORES):
        im = {"wts": _pack_wts(layout, layout["plans_b"], s),
              "xin": core_xin(s)}
        for j in range(N_SLOTS):
            im[f"ring0_{j}"] = r0s[j]
        in_maps.append(im)
    outsB = _run(ncB, in_maps, "phaseB")

    # ---- host combine (f64) ----
    # T for segment seg lives on core i slot j with slot_segs[j][i] == seg
    seg_loc = {}
    for j in range(N_SLOTS):
        for i in range(N_CORES):
            seg_loc[slot_segs[j][i]] = (i, j)
    wins = [np.zeros(WIN, np.float64)]
    for seg in range(N_SEG):
        i, j = seg_loc[seg]
        basis = nrhs_list[j] - 1
        T = outsB[i][f"tout_{j}"].astype(np.float64)
        T = T.transpose(1, 0, 2).reshape(WIN, basis + 1)
        w_next = T[:, :basis] @ wins[seg][WIN - basis:] + T[:, basis]
        wins.append(w_next)

    # ---- phase C: nrhs=1 with true initial windows ----
    ncC = _build_recur_nc(layout["plans_c"], [1] * N_SLOTS,
                          want_t=False, want_y=True)
    in_maps = []
    for s in range(N_CORES):
        im = {"wts": _pack_wts(layout, layout["plans_c"], s),
              "xin": core_xin(s)}
        for j in range(N_SLOTS):
            seg = slot_segs[j][s]
            w0 = wins[seg].astype(np.float16)
            im[f"ring0_{j}"] = np.ascontiguousarray(
                w0.reshape(4, CHUNK).T.reshape(CHUNK, 4, 1))
        in_maps.append(im)
    outsC = _run(ncC, in_maps, "phaseC")

    y = np.zeros(n, np.float32)
    for s in range(N_CORES):
        yo = outsC[s]["yout"]                        # [128, CPS*N_SLOTS]
        for j in range(N_SLOTS):
            seg = slot_segs[j][s]
            for c in range(CPS):
                y[(seg * CPS + c) * CHUNK:(seg * CPS + c + 1) * CHUNK] = \
                    yo[:, c * N_SLOTS + j]
    return y.astype(np.float32)


# revision 8
# speedup vs baseline: 1.8462x; 1.1745x over previous
"""Trainium2 Bass kernel for nn_DiffKS (differentiable Karplus-Strong).

Computation: y[t] = x[t] - sum_{j=0..5} vals[t,j] * y[t - 1 - z_l[t] - j],
vals / z_l from spline-interpolated delay & coefficient trajectories.
The feedback lag is always >= ~93, so 128-sample chunks are computed as
dense banded matmuls against a 512-sample window of past output (ring
columns in SBUF) plus a folded within-chunk correction (_fold_corr).

v2 strategy (32 segments, two SPMD programs, host combine between):
  - the 65536 samples are 512 chunks = 32 segments x 16 chunks; segments
    are grouped into 4 "slots" of 8 (one segment per core per slot),
    sorted by per-segment basis width so each slot's recurrence runs with
    the narrowest possible RHS count (nrhs = max initial-window reach of
    its segments + 1 particular column).
  - phase B (parallel): every core runs its 4 segments as 4 independent
    interleaved chunk-chains (hides the serial matmul->subtract chain and
    keeps the PE at max p-state) with nrhs_j columns: identity window
    basis + the excitation particular column.  Everything fp16 (weights,
    ring state, final-window transfer outputs); PE accumulates f32.
    Weights live SBUF-resident, prefetched with a few large
    partition-major DMAs.
  - combine (host, tiny, f64): chain the 32 transfer operators to get
    every segment's true initial window.
  - phase C (parallel): same chained recurrence with nrhs=1 and the true
    initial windows, emitting the corrected outputs.
fp16 end-to-end rel err ~5e-4 (validated in simulation + hardware).
"""

import os
import numpy as np

import concourse.bacc as bacc
import concourse.tile as tile
import concourse.mybir as mybir
from concourse.bass_utils import run_bass_kernel_spmd


def _ensure_ntff_hook():
    """The agent image's `antenv` stub lacks `axon_hooks`, which
    `run_bass_kernel_spmd(trace=True)` needs under axon for NTFF capture.
    Recreate the ctypes-based hook `trn_agent_boot.trn_boot` would install
    on images where the module exists."""
    try:
        from antenv.axon_hooks import get_axon_ntff_profile_hook  # noqa: F401
        return
    except ImportError:
        pass
    import contextlib
    import ctypes
    import sys
    import types

    so_path = "/opt/axon/libaxon_pjrt.so"
    if not os.path.exists(so_path):
        return
    lib = ctypes.CDLL(so_path)
    if not hasattr(lib, "axon_start_nrt_profile"):
        return
    lib.axon_start_nrt_profile.argtypes = [
        ctypes.POINTER(ctypes.c_int64), ctypes.c_size_t]
    lib.axon_start_nrt_profile.restype = ctypes.c_int64
    lib.axon_stop_nrt_profile.argtypes = [ctypes.c_char_p]
    lib.axon_stop_nrt_profile.restype = ctypes.c_int64

    @contextlib.contextmanager
    def _hook(output_dir, device_ids):
        import jax
        jax.devices()
        if device_ids:
            ids = (ctypes.c_int64 * len(device_ids))(*device_ids)
            rc = lib.axon_start_nrt_profile(ids, len(device_ids))
        else:
            rc = lib.axon_start_nrt_profile(None, 0)
        if rc != 0:
            raise RuntimeError(f"axon_start_nrt_profile rc={rc}")
        try:
            yield
        finally:
            n = lib.axon_stop_nrt_profile(str(output_dir).encode())
            if n <= 0:
                print(f"ntff profile: {n} file(s) written to {output_dir}",
                      file=sys.stderr)

    mod = types.ModuleType("antenv.axon_hooks")
    mod._hook = _hook
    mod.get_axon_ntff_profile_hook = lambda: _hook
    mod.set_axon_ntff_profile_hook = lambda h: setattr(mod, "_hook", h)
    import antenv
    antenv.axon_hooks = mod
    sys.modules["antenv.axon_hooks"] = mod


_ensure_ntff_hook()

F32 = mybir.dt.float32
F16 = mybir.dt.float16

N_SAMPLES = 65536
N_FRAMES = 64
L_ORDER = 5
CHUNK = 128
WIN = 512            # window length the chunk matmuls see (4 ring cols)
RING = 8             # ring columns in SBUF (power of two, >= 5)
CORR = 64            # within-chunk correction width (needs z_l >= 63)
N_CORES = 8
N_SLOTS = 4
CPS = 16             # chunks per segment
N_SEG = N_SLOTS * N_CORES
W_DMA_BLOCKS = 28    # weight blocks per prefetch DMA

# filled by kernel() with per-phase profiling results for the test harness
LAST_RESULTS = {}

_NC_CACHE = {}


# ----------------------------------------------------------------------------
# host-side preprocessing
# ----------------------------------------------------------------------------

_SPLINE_CACHE = {}


def _spline_matrix(n_in, n_out):
    """Static [n_out, n_in] natural-cubic-spline interpolation matrix for
    uniform knots (input-independent)."""
    key = (n_in, n_out)
    if key in _SPLINE_CACHE:
        return _SPLINE_CACHE[key]
    t_in = np.linspace(0.0, 1.0, n_in)
    t_out = np.linspace(0.0, 1.0, n_out)
    n = n_in
    h = t_in[1:] - t_in[:-1]
    R = np.zeros((n - 2, n))
    for i in range(n - 2):
        R[i, i] += 6.0 / h[i]
        R[i, i + 1] += -6.0 / h[i] - 6.0 / h[i + 1]
        R[i, i + 2] += 6.0 / h[i + 1]
    A = (
        np.diag(2.0 * (h[:-1] + h[1:]))
        + np.diag(h[1:-1], 1)
        + np.diag(h[1:-1], -1)
    )
    M = np.zeros((n, n))
    M[1:-1] = np.linalg.solve(A, R)
    idx = np.clip(np.searchsorted(t_in, t_out, side="right") - 1, 0, n - 2)
    dt = t_out - t_in[idx]
    S = np.zeros((n_out, n))
    eye = np.eye(n)
    for r in range(n_out):
        i = idx[r]
        b = (eye[i + 1] - eye[i]) / h[i] - h[i] * (2.0 * M[i] + M[i + 1]) / 6.0
        c = M[i] / 2.0
        d = (M[i + 1] - M[i]) / (6.0 * h[i])
        S[r] = eye[i] + b * dt[r] + c * dt[r] ** 2 + d * dt[r] ** 3
    S = S.astype(np.float32)
    _SPLINE_CACHE[key] = S
    return S


def _preprocess(delay, raw, exc, n_samples):
    sig = 1.0 / (1.0 + np.exp(-np.asarray(raw, np.float32)))
    coeff = sig / sig.sum(-1, keepdims=True)
    S = _spline_matrix(N_FRAMES, n_samples)
    delay_interp = S @ np.asarray(delay, np.float32)
    coeff_interp = S @ coeff
    z_l = np.floor(delay_interp).astype(np.int32)
    alfa = (delay_interp - z_l).astype(np.float32)
    b = coeff_interp
    v0 = -(1.0 - alfa) * b[:, 0]
    vmid = -(alfa[:, None] * b[:, : L_ORDER - 1]
             + (1.0 - alfa)[:, None] * b[:, 1:L_ORDER])
    vL = -alfa * b[:, -1]
    vals = np.concatenate([v0[:, None], vmid, vL[:, None]], 1).astype(np.float32)
    x = np.zeros(n_samples, np.float32)
    exc = np.asarray(exc, np.float32)
    x[: exc.shape[0]] = exc
    return vals, z_l, x


def _build_wts(vals, z_l, n_samples):
    """Dense per-chunk matmul weights in lhsT layout (see v1 docstring).
    wts [n_chunks, 5*128, 128]: groups 0..3 = window blocks, 4 = within-
    chunk correction block."""
    n_chunks = n_samples // CHUNK
    t = np.arange(n_samples)
    lag = 1 + z_l[:, None] + np.arange(6)[None, :]
    assert (lag[:, 0] >= CORR).all()
    basis = int(lag.max())
    assert basis <= WIN - CORR
    src = t[:, None] - lag
    i_in_chunk = t % CHUNK
    k_win = WIN + i_in_chunk[:, None] - lag
    wts = np.zeros((n_chunks, 5 * CHUNK, CHUNK), np.float32)
    c_of_t = t // CHUNK
    for j in range(6):
        valid = src[:, j] >= 0
        kw = k_win[:, j]
        in_window = valid & (kw < WIN)
        tw = t[in_window]
        wts[c_of_t[tw], kw[tw], i_in_chunk[tw]] += vals[tw, j]
        in_chunk = valid & (kw >= WIN)
        tc = t[in_chunk]
        kc = kw[tc] - WIN
        assert (kc < CORR).all()
        wts[c_of_t[tc], WIN + kc, i_in_chunk[tc]] += vals[tc, j]
    return wts, basis


def _fold_corr(wts_seg):
    """Fold each chunk's within-chunk correction into its in-segment
    readers so ring columns can stay uncorrected (exact algebra)."""
    wts_seg = wts_seg.copy()
    n = wts_seg.shape[0]
    blocks = wts_seg.reshape(n, 5, CHUNK, CHUNK)
    corr_active = np.abs(blocks[:, 4]).reshape(n, -1).max(-1) > 0
    for w in range(n):
        if not corr_active[w]:
            continue
        corrT = blocks[w, 4]
        for r in range(w + 1, min(w + 5, n)):
            g = w - r + 4
            blk = blocks[r, g]
            blk[0:CORR] -= corrT[0:CORR, CORR:] @ blk[CORR:]
    return wts_seg


def _segment_layout(vals, z_l, wts):
    """Slot assignment + per-slot plans.

    Returns dict with:
      slot_segs [N_SLOTS][N_CORES] -> segment index (time order 0..31)
      nrhs      [N_SLOTS]          -> basis_j + 1
      plans_b / plans_c: [N_SLOTS][CPS] -> (window_gs, corr_flag)
      seg_fold  [N_SEG] -> folded weights [CPS, 5*128, 128] f32
    """
    lag = 1 + z_l[:, None] + np.arange(6)[None, :]
    t = np.arange(N_SAMPLES)
    seglen = CPS * CHUNK
    bas = []
    for s in range(N_SEG):
        t0 = s * seglen
        reach = lag[t0:t0 + seglen] - (t[t0:t0 + seglen] - t0)[:, None]
        bas.append(int(reach.max()))
    bas = np.array(bas)
    order = np.argsort(bas, kind="stable")
    slot_segs = [order[8 * j: 8 * j + 8].tolist() for j in range(N_SLOTS)]
    nrhs = [int(bas[g].max()) + 1 for g in slot_segs]

    seg_fold = [_fold_corr(wts[s * CPS:(s + 1) * CPS]) for s in range(N_SEG)]
    act = np.stack([
        np.abs(f.reshape(CPS, 5, -1)).max(-1) > 0 for f in seg_fold
    ])                                            # [N_SEG, CPS, 5]
    plans_b, plans_c = [], []
    for j in range(N_SLOTS):
        u = act[slot_segs[j]].any(0)              # [CPS, 5]
        pb, pcn = [], []
        for c in range(CPS):
            wb = [g for g in range(4) if u[c, g]]
            if not wb:
                wb = [3]
            pb.append((wb, bool(u[c, 4] and c >= CPS - 4)))
            pcn.append((wb, bool(u[c, 4])))
        plans_b.append(pb)
        plans_c.append(pcn)
    xslot = next(j for j in range(N_SLOTS) if 0 in slot_segs[j])
    xcore = slot_segs[xslot].index(0)
    return dict(slot_segs=slot_segs, nrhs=nrhs, basis=[n - 1 for n in nrhs],
                plans_b=plans_b, plans_c=plans_c, seg_fold=seg_fold, bas=bas,
                xslot=xslot, xcore=xcore)


def _pack_wts(layout, plans, core):
    """Per-core packed fp16 weights, partition-major [128, NB, 128],
    in emission order: for c in 0..CPS-1: for j in 0..N_SLOTS-1:
    window blocks then (if flagged) the correction block."""
    out = []
    for c in range(CPS):
        for j in range(N_SLOTS):
            seg = layout["slot_segs"][j][core]
            blocks = layout["seg_fold"][seg].reshape(CPS, 5, CHUNK, CHUNK)
            wb, co = plans[j][c]
            sel = list(wb) + ([4] if co else [])
            out.append(blocks[c, sel])
    packed = -np.concatenate(out, 0)              # [NB, 128, 128], sign-
    return np.ascontiguousarray(                      # folded: ring = +psum
        packed.transpose(1, 0, 2)).astype(np.float16)  # [128, NB, 128]


def _plan_nblocks(plans):
    return sum(len(wb) + int(co)
               for j in range(N_SLOTS) for wb, co in plans[j])


# ----------------------------------------------------------------------------
# bass program builder
# ----------------------------------------------------------------------------

def _plan_key(plans):
    return tuple(tuple((tuple(wb), co) for wb, co in p) for p in plans)


def _build_recur_nc(plans, nrhs_list, xslot, want_t, want_y):
    key = ("recur3", _plan_key(plans), tuple(nrhs_list), xslot, want_t, want_y)
    if key in _NC_CACHE:
        return _NC_CACHE[key]
    nc = _build_recur_nc_impl(plans, nrhs_list, xslot, want_t, want_y)
    _NC_CACHE[key] = nc
    return nc


def _build_recur_nc_impl(plans, nrhs_list, xslot, want_t, want_y):
    """Chained chunk recurrence over N_SLOTS independent interleaved
    chains (one segment per slot per core), nrhs_list[j] RHS columns.

    Weights are sign-folded on the host (packed = -W), so each chunk is:
        psum = sum_g (-W_g)^T ring_col_g  (+ x * e_basis at x positions)
        ring_col = copy(psum)             (fp16, on a rotated engine)
    The excitation is nonzero only in the first 4 chunks of segment 0, so
    x enters via a rank-1 PE matmul at those 4 (slot xslot) positions
    instead of a per-position vector op.

    Inputs:  wts   [128, NB, 128] f16  (sign-folded packed lhsT blocks)
             xinT  [1, 4*CHUNK]    f16 (x of chunks 0..3; zero on cores
                                        not owning segment 0)
             ebas  [1, nrhs_xslot] f16 (one-hot at the particular column)
             ring0_j [128, 4, nrhs_j] f16 per slot (initial window)
    Outputs: tout_j [128, 4, nrhs_j] f16 per slot (true final window)
             yout  [128, CPS * N_SLOTS] f32 (corrected outputs)
    """
    NB = _plan_nblocks(plans)
    nc = bacc.Bacc("TRN2", target_bir_lowering=False, debug=False,
                   num_devices=N_CORES, enable_partition_id=False)
    wts = nc.dram_tensor("wts", [CHUNK, NB, CHUNK], F16, kind="ExternalInput")
    xinT = nc.dram_tensor("xinT", [1, 4 * CHUNK], F16, kind="ExternalInput")
    ebas = nc.dram_tensor("ebas", [1, nrhs_list[xslot]], F16,
                          kind="ExternalInput")
    ring0 = [nc.dram_tensor(f"ring0_{j}", [CHUNK, 4, nrhs_list[j]], F16,
                            kind="ExternalInput") for j in range(N_SLOTS)]
    tout = yout = None
    if want_t:
        tout = [nc.dram_tensor(f"tout_{j}", [CHUNK, 4, nrhs_list[j]], F16,
                               kind="ExternalOutput") for j in range(N_SLOTS)]
    if want_y:
        yout = nc.dram_tensor("yout", [CHUNK, CPS * N_SLOTS], F32,
                              kind="ExternalOutput")

    with tile.TileContext(nc) as tc:
        with (
            tc.tile_pool(name="state", bufs=1) as state,
            tc.tile_pool(name="stpool", bufs=2) as stpool,
            tc.tile_pool(name="psum", bufs=1, space="PSUM") as ppool,
        ):
            wsb = state.tile([CHUNK, NB, CHUNK], F16)
            rings = [state.tile([CHUNK, RING, nrhs_list[j]], F16,
                                name=f"ring{j}")
                     for j in range(N_SLOTS)]
            xinT_sb = state.tile([1, 4 * CHUNK], F16)
            ebas_sb = state.tile([1, nrhs_list[xslot]], F16)
            for j in range(N_SLOTS):
                nc.sync.dma_start(rings[j][:, 4:8, :], ring0[j][:])
            nc.sync.dma_start(xinT_sb[:], xinT[:])
            nc.sync.dma_start(ebas_sb[:], ebas[:])
            # weights in emission-order runs; small first run so the first
            # matmuls unblock quickly
            bnds = [0, min(6, NB)]
            while bnds[-1] < NB:
                bnds.append(min(bnds[-1] + W_DMA_BLOCKS, NB))
            for a, b in zip(bnds[:-1], bnds[1:]):
                nc.sync.dma_start(wsb[:, a:b, :], wts[:, a:b, :])

            yout_sb = None
            if want_y:
                yout_sb = state.tile([CHUNK, CPS * N_SLOTS], F32)

            # psum->ring copy engines, rotated per slot (vector = DVE,
            # scalar = ACT via activation-Copy, gpsimd as third lane)
            # PSUM-reading ops go on vector/scalar only; gpsimd handles
            # SBUF-to-SBUF copies
            def ring_copy(j, dst, src):
                if j % 2 == 0:
                    nc.vector.tensor_copy(dst, src)
                else:
                    nc.scalar.activation(
                        dst, src, mybir.ActivationFunctionType.Copy)

            def aux_copy(j, dst, src):
                e = j % 3
                if e == 0:
                    nc.gpsimd.tensor_copy(dst, src)
                elif e == 1:
                    nc.vector.tensor_copy(dst, src)
                else:
                    nc.scalar.activation(
                        dst, src, mybir.ActivationFunctionType.Copy)

            def aux_add(j, dst, a, b):
                nc.vector.tensor_add(dst, a, b)

            lo = slice(0, CORR)
            hi = slice(CORR, CHUNK)
            off = 0
            for c in range(CPS):
                for j in range(N_SLOTS):
                    nrhs = nrhs_list[j]
                    ring = rings[j]
                    wb, corr = plans[j][c]
                    xact = (j == xslot and c < 4)
                    pos = c * N_SLOTS + j
                    n_acc = len(wb) + int(xact)
                    psum = ppool.tile([CHUNK, nrhs], F32, tag=f"acc{j}")
                    for i, g in enumerate(wb):
                        col = (c + 4 + g) % RING
                        nc.tensor.matmul(
                            psum[:],
                            wsb[:, off + i, :],
                            ring[:, col, :],
                            start=(i == 0),
                            stop=(i == n_acc - 1),
                        )
                    if xact:
                        nc.tensor.matmul(
                            psum[:],
                            xinT_sb[0:1, c * CHUNK:(c + 1) * CHUNK],
                            ebas_sb[0:1, :],
                            start=False,
                            stop=True,
                        )
                    rc = c % RING
                    ring_copy(j, ring[:, rc, :], psum[:])
                    psum2 = None
                    if corr:
                        psum2 = ppool.tile([CHUNK, nrhs], F32, tag=f"corr{j}")
                        nc.tensor.matmul(
                            psum2[:],
                            wsb[lo, off + len(wb), :],
                            ring[lo, rc, :],
                            start=True,
                            stop=True,
                        )
                    off += len(wb) + int(corr)
                    if want_y:
                        if corr:
                            aux_copy(j, yout_sb[lo, pos: pos + 1],
                                     ring[lo, rc, :])
                            aux_add(j, yout_sb[hi, pos: pos + 1],
                                    ring[hi, rc, :], psum2[hi, :])
                        else:
                            aux_copy(j, yout_sb[:, pos: pos + 1],
                                     ring[:, rc, :])
                    if want_t and c >= CPS - 4:
                        k = c - (CPS - 4)
                        if corr:
                            stage = stpool.tile([CHUNK, nrhs], F16,
                                                tag=f"stage{j}")
                            aux_add(j, stage[hi, :], ring[hi, rc, :],
                                    psum2[hi, :])
                            nc.sync.dma_start(
                                tout[j][lo, k, :], ring[lo, rc, :])
                            nc.sync.dma_start(
                                tout[j][hi, k, :], stage[hi, :])
                        else:
                            nc.sync.dma_start(
                                tout[j][:, k, :], ring[:, rc, :])
            assert off == NB

            if want_y:
                nc.sync.dma_start(yout[:], yout_sb[:])
    nc.compile()
    return nc


# ----------------------------------------------------------------------------
# host orchestration
# ----------------------------------------------------------------------------

def _run(nc, in_maps, tag):
    trace = bool(int(os.environ.get("DIFFKS_TRACE", "0")))
    kw = {}
    tcs = os.environ.get("DIFFKS_TRACE_CORES", "")
    if trace and tcs:
        kw["trace_cores"] = [int(x) for x in tcs.split(",")]
    res = run_bass_kernel_spmd(
        nc, in_maps, core_ids=list(range(len(in_maps))), trace=trace, **kw
    )
    LAST_RESULTS[tag] = res
    return res.results


def _basis_ring0(basis):
    """Initial window columns for phase B: basis b is a unit vector at
    window position (WIN-basis)+b; the particular column starts at zero."""
    nrhs = basis + 1
    r0 = np.zeros((CHUNK, 4, nrhs), np.float16)
    for b in range(basis):
        p = (WIN - basis) + b
        r0[p % CHUNK, p // CHUNK, b] = 1.0
    return r0


def kernel(delay_len_frames, raw_coeff_frames, excitation, n_samples):
    n = int(n_samples)
    assert n == N_SAMPLES, f"kernel hardcoded for {N_SAMPLES}, got {n}"
    LAST_RESULTS.clear()

    vals, z_l, x = _preprocess(delay_len_frames, raw_coeff_frames,
                               excitation, n)
    wts, _ = _build_wts(vals, z_l, n)
    layout = _segment_layout(vals, z_l, wts)
    slot_segs = layout["slot_segs"]
    nrhs_list = layout["nrhs"]
    xslot, xcore = layout["xslot"], layout["xcore"]

    def core_xinT(core):
        xi = np.zeros((1, 4 * CHUNK), np.float16)
        if core == xcore:
            xi[0, :] = x[:4 * CHUNK].astype(np.float16)
        return xi

    # ---- phase B ----
    ncB = _build_recur_nc(layout["plans_b"], nrhs_list, xslot,
                          want_t=True, want_y=False)
    ebasB = np.zeros((1, nrhs_list[xslot]), np.float16)
    ebasB[0, -1] = 1.0
    r0s = [_basis_ring0(nrhs_list[j] - 1) for j in range(N_SLOTS)]
    in_maps = []
    for s in range(N_CORES):
        im = {"wts": _pack_wts(layout, layout["plans_b"], s),
              "xinT": core_xinT(s), "ebas": ebasB}
        for j in range(N_SLOTS):
            im[f"ring0_{j}"] = r0s[j]
        in_maps.append(im)
    outsB = _run(ncB, in_maps, "phaseB")

    # ---- host combine (f64) ----
    seg_loc = {}
    for j in range(N_SLOTS):
        for i in range(N_CORES):
            seg_loc[slot_segs[j][i]] = (i, j)
    wins = [np.zeros(WIN, np.float64)]
    for seg in range(N_SEG):
        i, j = seg_loc[seg]
        basis = nrhs_list[j] - 1
        T = outsB[i][f"tout_{j}"].astype(np.float64)
        T = T.transpose(1, 0, 2).reshape(WIN, basis + 1)
        w_next = T[:, :basis] @ wins[seg][WIN - basis:] + T[:, basis]
        wins.append(w_next)

    # ---- phase C: nrhs=1 with true initial windows ----
    ncC = _build_recur_nc(layout["plans_c"], [1] * N_SLOTS, xslot,
                          want_t=False, want_y=True)
    ebasC = np.ones((1, 1), np.float16)
    in_maps = []
    for s in range(N_CORES):
        im = {"wts": _pack_wts(layout, layout["plans_c"], s),
              "xinT": core_xinT(s), "ebas": ebasC}
        for j in range(N_SLOTS):
            seg = slot_segs[j][s]
            w0 = wins[seg].astype(np.float16)
            im[f"ring0_{j}"] = np.ascontiguousarray(
                w0.reshape(4, CHUNK).T.reshape(CHUNK, 4, 1))
        in_maps.append(im)
    outsC = _run(ncC, in_maps, "phaseC")

    y = np.zeros(n, np.float32)
    for s in range(N_CORES):
        yo = outsC[s]["yout"]                        # [128, CPS*N_SLOTS]
        for j in range(N_SLOTS):
            seg = slot_segs[j][s]
            for c in range(CPS):
                y[(seg * CPS + c) * CHUNK:(seg * CPS + c + 1) * CHUNK] = \
                    yo[:, c * N_SLOTS + j]
    return y.astype(np.float32)


# revision 9
# speedup vs baseline: 1.8537x; 1.0040x over previous
"""Trainium2 Bass kernel for nn_DiffKS (differentiable Karplus-Strong).

Computation: y[t] = x[t] - sum_{j=0..5} vals[t,j] * y[t - 1 - z_l[t] - j],
vals / z_l from spline-interpolated delay & coefficient trajectories.
The feedback lag is always >= ~93, so 128-sample chunks are computed as
dense banded matmuls against a 512-sample window of past output (ring
columns in SBUF) plus a folded within-chunk correction (_fold_corr).

v2 strategy (32 segments, two SPMD programs, host combine between):
  - the 65536 samples are 512 chunks = 32 segments x 16 chunks; segments
    are grouped into 4 "slots" of 8 (one segment per core per slot),
    sorted by per-segment basis width so each slot's recurrence runs with
    the narrowest possible RHS count (nrhs = max initial-window reach of
    its segments + 1 particular column).
  - phase B (parallel): every core runs its 4 segments as 4 independent
    interleaved chunk-chains (hides the serial matmul->subtract chain and
    keeps the PE at max p-state) with nrhs_j columns: identity window
    basis + the excitation particular column.  Everything fp16 (weights,
    ring state, final-window transfer outputs); PE accumulates f32.
    Weights live SBUF-resident, prefetched with a few large
    partition-major DMAs.
  - combine (host, tiny, f64): chain the 32 transfer operators to get
    every segment's true initial window.
  - phase C (parallel): same chained recurrence with nrhs=1 and the true
    initial windows, emitting the corrected outputs.
fp16 end-to-end rel err ~5e-4 (validated in simulation + hardware).
"""

import os
import numpy as np

import concourse.bacc as bacc
import concourse.tile as tile
import concourse.mybir as mybir
from concourse.bass_utils import run_bass_kernel_spmd


def _ensure_ntff_hook():
    """The agent image's `antenv` stub lacks `axon_hooks`, which
    `run_bass_kernel_spmd(trace=True)` needs under axon for NTFF capture.
    Recreate the ctypes-based hook `trn_agent_boot.trn_boot` would install
    on images where the module exists."""
    try:
        from antenv.axon_hooks import get_axon_ntff_profile_hook  # noqa: F401
        return
    except ImportError:
        pass
    import contextlib
    import ctypes
    import sys
    import types

    so_path = "/opt/axon/libaxon_pjrt.so"
    if not os.path.exists(so_path):
        return
    lib = ctypes.CDLL(so_path)
    if not hasattr(lib, "axon_start_nrt_profile"):
        return
    lib.axon_start_nrt_profile.argtypes = [
        ctypes.POINTER(ctypes.c_int64), ctypes.c_size_t]
    lib.axon_start_nrt_profile.restype = ctypes.c_int64
    lib.axon_stop_nrt_profile.argtypes = [ctypes.c_char_p]
    lib.axon_stop_nrt_profile.restype = ctypes.c_int64

    @contextlib.contextmanager
    def _hook(output_dir, device_ids):
        import jax
        jax.devices()
        if device_ids:
            ids = (ctypes.c_int64 * len(device_ids))(*device_ids)
            rc = lib.axon_start_nrt_profile(ids, len(device_ids))
        else:
            rc = lib.axon_start_nrt_profile(None, 0)
        if rc != 0:
            raise RuntimeError(f"axon_start_nrt_profile rc={rc}")
        try:
            yield
        finally:
            n = lib.axon_stop_nrt_profile(str(output_dir).encode())
            if n <= 0:
                print(f"ntff profile: {n} file(s) written to {output_dir}",
                      file=sys.stderr)

    mod = types.ModuleType("antenv.axon_hooks")
    mod._hook = _hook
    mod.get_axon_ntff_profile_hook = lambda: _hook
    mod.set_axon_ntff_profile_hook = lambda h: setattr(mod, "_hook", h)
    import antenv
    antenv.axon_hooks = mod
    sys.modules["antenv.axon_hooks"] = mod


_ensure_ntff_hook()

F32 = mybir.dt.float32
F16 = mybir.dt.float16

N_SAMPLES = 65536
N_FRAMES = 64
L_ORDER = 5
CHUNK = 128
WIN = 512            # window length the chunk matmuls see (4 ring cols)
RING = 8             # ring columns in SBUF (power of two, >= 5)
CORR = 64            # within-chunk correction width (needs z_l >= 63)
N_CORES = 8
N_SLOTS = 4
CPS = 16             # chunks per segment
N_SEG = N_SLOTS * N_CORES
W_DMA_BLOCKS = 28    # weight blocks per prefetch DMA

# filled by kernel() with per-phase profiling results for the test harness
LAST_RESULTS = {}

_NC_CACHE = {}


# ----------------------------------------------------------------------------
# host-side preprocessing
# ----------------------------------------------------------------------------

_SPLINE_CACHE = {}


def _spline_matrix(n_in, n_out):
    """Static [n_out, n_in] natural-cubic-spline interpolation matrix for
    uniform knots (input-independent)."""
    key = (n_in, n_out)
    if key in _SPLINE_CACHE:
        return _SPLINE_CACHE[key]
    t_in = np.linspace(0.0, 1.0, n_in)
    t_out = np.linspace(0.0, 1.0, n_out)
    n = n_in
    h = t_in[1:] - t_in[:-1]
    R = np.zeros((n - 2, n))
    for i in range(n - 2):
        R[i, i] += 6.0 / h[i]
        R[i, i + 1] += -6.0 / h[i] - 6.0 / h[i + 1]
        R[i, i + 2] += 6.0 / h[i + 1]
    A = (
        np.diag(2.0 * (h[:-1] + h[1:]))
        + np.diag(h[1:-1], 1)
        + np.diag(h[1:-1], -1)
    )
    M = np.zeros((n, n))
    M[1:-1] = np.linalg.solve(A, R)
    idx = np.clip(np.searchsorted(t_in, t_out, side="right") - 1, 0, n - 2)
    dt = t_out - t_in[idx]
    S = np.zeros((n_out, n))
    eye = np.eye(n)
    for r in range(n_out):
        i = idx[r]
        b = (eye[i + 1] - eye[i]) / h[i] - h[i] * (2.0 * M[i] + M[i + 1]) / 6.0
        c = M[i] / 2.0
        d = (M[i + 1] - M[i]) / (6.0 * h[i])
        S[r] = eye[i] + b * dt[r] + c * dt[r] ** 2 + d * dt[r] ** 3
    S = S.astype(np.float32)
    _SPLINE_CACHE[key] = S
    return S


def _preprocess(delay, raw, exc, n_samples):
    sig = 1.0 / (1.0 + np.exp(-np.asarray(raw, np.float32)))
    coeff = sig / sig.sum(-1, keepdims=True)
    S = _spline_matrix(N_FRAMES, n_samples)
    delay_interp = S @ np.asarray(delay, np.float32)
    coeff_interp = S @ coeff
    z_l = np.floor(delay_interp).astype(np.int32)
    alfa = (delay_interp - z_l).astype(np.float32)
    b = coeff_interp
    v0 = -(1.0 - alfa) * b[:, 0]
    vmid = -(alfa[:, None] * b[:, : L_ORDER - 1]
             + (1.0 - alfa)[:, None] * b[:, 1:L_ORDER])
    vL = -alfa * b[:, -1]
    vals = np.concatenate([v0[:, None], vmid, vL[:, None]], 1).astype(np.float32)
    x = np.zeros(n_samples, np.float32)
    exc = np.asarray(exc, np.float32)
    x[: exc.shape[0]] = exc
    return vals, z_l, x


def _build_wts(vals, z_l, n_samples):
    """Dense per-chunk matmul weights in lhsT layout (see v1 docstring).
    wts [n_chunks, 5*128, 128]: groups 0..3 = window blocks, 4 = within-
    chunk correction block."""
    n_chunks = n_samples // CHUNK
    t = np.arange(n_samples)
    lag = 1 + z_l[:, None] + np.arange(6)[None, :]
    assert (lag[:, 0] >= CORR).all()
    basis = int(lag.max())
    assert basis <= WIN - CORR
    src = t[:, None] - lag
    i_in_chunk = t % CHUNK
    k_win = WIN + i_in_chunk[:, None] - lag
    wts = np.zeros((n_chunks, 5 * CHUNK, CHUNK), np.float32)
    c_of_t = t // CHUNK
    for j in range(6):
        valid = src[:, j] >= 0
        kw = k_win[:, j]
        in_window = valid & (kw < WIN)
        tw = t[in_window]
        wts[c_of_t[tw], kw[tw], i_in_chunk[tw]] += vals[tw, j]
        in_chunk = valid & (kw >= WIN)
        tc = t[in_chunk]
        kc = kw[tc] - WIN
        assert (kc < CORR).all()
        wts[c_of_t[tc], WIN + kc, i_in_chunk[tc]] += vals[tc, j]
    return wts, basis


def _fold_corr(wts_seg):
    """Fold each chunk's within-chunk correction into its in-segment
    readers so ring columns can stay uncorrected (exact algebra)."""
    wts_seg = wts_seg.copy()
    n = wts_seg.shape[0]
    blocks = wts_seg.reshape(n, 5, CHUNK, CHUNK)
    corr_active = np.abs(blocks[:, 4]).reshape(n, -1).max(-1) > 0
    for w in range(n):
        if not corr_active[w]:
            continue
        corrT = blocks[w, 4]
        for r in range(w + 1, min(w + 5, n)):
            g = w - r + 4
            blk = blocks[r, g]
            blk[0:CORR] -= corrT[0:CORR, CORR:] @ blk[CORR:]
    return wts_seg


def _segment_layout(vals, z_l, wts):
    """Slot assignment + per-slot plans.

    Returns dict with:
      slot_segs [N_SLOTS][N_CORES] -> segment index (time order 0..31)
      nrhs      [N_SLOTS]          -> basis_j + 1
      plans_b / plans_c: [N_SLOTS][CPS] -> (window_gs, corr_flag)
      seg_fold  [N_SEG] -> folded weights [CPS, 5*128, 128] f32
    """
    lag = 1 + z_l[:, None] + np.arange(6)[None, :]
    t = np.arange(N_SAMPLES)
    seglen = CPS * CHUNK
    bas = []
    for s in range(N_SEG):
        t0 = s * seglen
        reach = lag[t0:t0 + seglen] - (t[t0:t0 + seglen] - t0)[:, None]
        bas.append(int(reach.max()))
    bas = np.array(bas)
    order = np.argsort(bas, kind="stable")
    slot_segs = [order[8 * j: 8 * j + 8].tolist() for j in range(N_SLOTS)]
    nrhs = [int(bas[g].max()) + 1 for g in slot_segs]

    seg_fold = [_fold_corr(wts[s * CPS:(s + 1) * CPS]) for s in range(N_SEG)]
    act = np.stack([
        np.abs(f.reshape(CPS, 5, -1)).max(-1) > 0 for f in seg_fold
    ])                                            # [N_SEG, CPS, 5]
    plans_b, plans_c = [], []
    for j in range(N_SLOTS):
        u = act[slot_segs[j]].any(0)              # [CPS, 5]
        pb, pcn = [], []
        for c in range(CPS):
            wb = [g for g in range(4) if u[c, g]]
            if not wb:
                wb = [3]
            pb.append((wb, bool(u[c, 4] and c >= CPS - 4)))
            pcn.append((wb, bool(u[c, 4])))
        plans_b.append(pb)
        plans_c.append(pcn)
    xslot = next(j for j in range(N_SLOTS) if 0 in slot_segs[j])
    xcore = slot_segs[xslot].index(0)
    return dict(slot_segs=slot_segs, nrhs=nrhs, basis=[n - 1 for n in nrhs],
                plans_b=plans_b, plans_c=plans_c, seg_fold=seg_fold, bas=bas,
                xslot=xslot, xcore=xcore)


def _pack_wts(layout, plans, core):
    """Per-core packed fp16 weights, partition-major [128, NB, 128],
    in emission order: for c in 0..CPS-1: for j in 0..N_SLOTS-1:
    window blocks then (if flagged) the correction block."""
    out = []
    for c in range(CPS):
        for j in range(N_SLOTS):
            seg = layout["slot_segs"][j][core]
            blocks = layout["seg_fold"][seg].reshape(CPS, 5, CHUNK, CHUNK)
            wb, co = plans[j][c]
            sel = list(wb) + ([4] if co else [])
            out.append(blocks[c, sel])
    packed = -np.concatenate(out, 0)              # [NB, 128, 128], sign-
    return np.ascontiguousarray(                      # folded: ring = +psum
        packed.transpose(1, 0, 2)).astype(np.float16)  # [128, NB, 128]


def _plan_nblocks(plans):
    return sum(len(wb) + int(co)
               for j in range(N_SLOTS) for wb, co in plans[j])


# ----------------------------------------------------------------------------
# bass program builder
# ----------------------------------------------------------------------------

def _plan_key(plans):
    return tuple(tuple((tuple(wb), co) for wb, co in p) for p in plans)


def _build_recur_nc(plans, nrhs_list, xslot, want_t, want_y):
    key = ("recur4", _plan_key(plans), tuple(nrhs_list), xslot, want_t, want_y)
    if key in _NC_CACHE:
        return _NC_CACHE[key]
    nc = _build_recur_nc_impl(plans, nrhs_list, xslot, want_t, want_y)
    _NC_CACHE[key] = nc
    return nc


def _build_recur_nc_impl(plans, nrhs_list, xslot, want_t, want_y):
    """Chained chunk recurrence over N_SLOTS independent interleaved
    chains (one segment per slot per core), nrhs_list[j] RHS columns.

    Weights are sign-folded on the host (packed = -W), so each chunk is:
        psum = sum_g (-W_g)^T ring_col_g  (+ x * e_basis at x positions)
        ring_col = copy(psum)             (fp16, vector engine)
    The excitation is nonzero only in the first 4 chunks of segment 0, so
    x enters via a rank-1 PE matmul at those 4 (slot xslot) positions.
    All slot states share merged tiles (column offsets offs[j]) so inputs
    and outputs move with single DMAs; DMA issues are spread over the
    otherwise-idle scalar/gpsimd queues (each dma_start costs ~0.6us of
    issue time on its queue).

    Inputs:  wts  [128, NB, 128] f16  (sign-folded packed lhsT blocks)
             aux  [1, 4*CHUNK + nrhs_xslot] f16 (x chunks 0..3 || e_basis)
             ring0 [128, 4, S] f16    (initial windows, S = sum nrhs)
    Outputs: tout [128, 4, S] f16     (true final windows, want_t)
             yout [128, CPS * N_SLOTS] f32 (corrected outputs, want_y)
    """
    NB = _plan_nblocks(plans)
    S = sum(nrhs_list)
    offs = [sum(nrhs_list[:j]) for j in range(N_SLOTS)]
    nx = nrhs_list[xslot]
    nc = bacc.Bacc("TRN2", target_bir_lowering=False, debug=False,
                   num_devices=N_CORES, enable_partition_id=False)
    wts = nc.dram_tensor("wts", [CHUNK, NB, CHUNK], F16, kind="ExternalInput")
    aux = nc.dram_tensor("aux", [1, 4 * CHUNK + nx], F16,
                         kind="ExternalInput")
    ring0 = nc.dram_tensor("ring0", [CHUNK, 4, S], F16, kind="ExternalInput")
    tout = yout = None
    if want_t:
        tout = nc.dram_tensor("tout", [CHUNK, 4, S], F16,
                              kind="ExternalOutput")
    if want_y:
        yout = nc.dram_tensor("yout", [CHUNK, CPS * N_SLOTS], F32,
                              kind="ExternalOutput")

    with tile.TileContext(nc) as tc:
        with (
            tc.tile_pool(name="state", bufs=1) as state,
            tc.tile_pool(name="psum", bufs=1, space="PSUM") as ppool,
        ):
            wsb = state.tile([CHUNK, NB, CHUNK], F16)
            rings = state.tile([CHUNK, RING, S], F16)
            aux_sb = state.tile([1, 4 * CHUNK + nx], F16)
            # first weight run gates the first matmuls: issue it first and
            # keep it small; remaining runs go wide on the scalar queue
            bnds = [0, min(6, NB)]
            while bnds[-1] < NB:
                bnds.append(min(bnds[-1] + 64, NB))
            nc.scalar.dma_start(wsb[:, 0:bnds[1], :], wts[:, 0:bnds[1], :])
            nc.gpsimd.dma_start(rings[:, 4:8, :], ring0[:])
            nc.gpsimd.dma_start(aux_sb[:], aux[:])
            for a, b in zip(bnds[1:-1], bnds[2:]):
                nc.scalar.dma_start(wsb[:, a:b, :], wts[:, a:b, :])

            trueout = None
            if want_t:
                trueout = state.tile([CHUNK, 4, S], F16)
            yout_sb = None
            if want_y:
                yout_sb = state.tile([CHUNK, CPS * N_SLOTS], F32)

            lo = slice(0, CORR)
            hi = slice(CORR, CHUNK)
            off = 0
            for c in range(CPS):
                for j in range(N_SLOTS):
                    nrhs = nrhs_list[j]
                    o0, o1 = offs[j], offs[j] + nrhs
                    wb, corr = plans[j][c]
                    xact = (j == xslot and c < 4)
                    pos = c * N_SLOTS + j
                    n_acc = len(wb) + int(xact)
                    psum = ppool.tile([CHUNK, nrhs], F32, tag=f"acc{j}")
                    for i, g in enumerate(wb):
                        col = (c + 4 + g) % RING
                        nc.tensor.matmul(
                            psum[:],
                            wsb[:, off + i, :],
                            rings[:, col, o0:o1],
                            start=(i == 0),
                            stop=(i == n_acc - 1),
                        )
                    if xact:
                        nc.tensor.matmul(
                            psum[:],
                            aux_sb[0:1, c * CHUNK:(c + 1) * CHUNK],
                            aux_sb[0:1, 4 * CHUNK:],
                            start=False,
                            stop=True,
                        )
                    rc = c % RING
                    nc.vector.tensor_copy(rings[:, rc, o0:o1], psum[:])
                    psum2 = None
                    if corr:
                        psum2 = ppool.tile([CHUNK, nrhs], F32, tag=f"corr{j}")
                        nc.tensor.matmul(
                            psum2[:],
                            wsb[lo, off + len(wb), :],
                            rings[lo, rc, o0:o1],
                            start=True,
                            stop=True,
                        )
                    off += len(wb) + int(corr)
                    if want_y:
                        if corr:
                            nc.gpsimd.tensor_copy(
                                yout_sb[lo, pos: pos + 1], rings[lo, rc, o0:o1])
                            nc.vector.tensor_add(
                                yout_sb[hi, pos: pos + 1],
                                rings[hi, rc, o0:o1], psum2[hi, :])
                        else:
                            nc.gpsimd.tensor_copy(
                                yout_sb[:, pos: pos + 1], rings[:, rc, o0:o1])
                    if want_t and c >= CPS - 4:
                        k = c - (CPS - 4)
                        if corr:
                            nc.gpsimd.tensor_copy(
                                trueout[lo, k, o0:o1], rings[lo, rc, o0:o1])
                            nc.vector.tensor_add(
                                trueout[hi, k, o0:o1],
                                rings[hi, rc, o0:o1], psum2[hi, :])
                        else:
                            nc.gpsimd.tensor_copy(
                                trueout[:, k, o0:o1], rings[:, rc, o0:o1])
            assert off == NB

            if want_t:
                nc.scalar.dma_start(tout[:], trueout[:])
            if want_y:
                nc.scalar.dma_start(yout[:], yout_sb[:])
    nc.compile()
    return nc


# ----------------------------------------------------------------------------
# host orchestration
# ----------------------------------------------------------------------------

def _run(nc, in_maps, tag):
    trace = bool(int(os.environ.get("DIFFKS_TRACE", "0")))
    kw = {}
    tcs = os.environ.get("DIFFKS_TRACE_CORES", "")
    if trace and tcs:
        kw["trace_cores"] = [int(x) for x in tcs.split(",")]
    res = run_bass_kernel_spmd(
        nc, in_maps, core_ids=list(range(len(in_maps))), trace=trace, **kw
    )
    LAST_RESULTS[tag] = res
    return res.results


def _basis_ring0(basis):
    """Initial window columns for phase B: basis b is a unit vector at
    window position (WIN-basis)+b; the particular column starts at zero."""
    nrhs = basis + 1
    r0 = np.zeros((CHUNK, 4, nrhs), np.float16)
    for b in range(basis):
        p = (WIN - basis) + b
        r0[p % CHUNK, p // CHUNK, b] = 1.0
    return r0


def kernel(delay_len_frames, raw_coeff_frames, excitation, n_samples):
    n = int(n_samples)
    assert n == N_SAMPLES, f"kernel hardcoded for {N_SAMPLES}, got {n}"
    LAST_RESULTS.clear()

    vals, z_l, x = _preprocess(delay_len_frames, raw_coeff_frames,
                               excitation, n)
    wts, _ = _build_wts(vals, z_l, n)
    layout = _segment_layout(vals, z_l, wts)
    slot_segs = layout["slot_segs"]
    nrhs_list = layout["nrhs"]
    xslot, xcore = layout["xslot"], layout["xcore"]

    def core_xinT(core):
        xi = np.zeros((1, 4 * CHUNK), np.float16)
        if core == xcore:
            xi[0, :] = x[:4 * CHUNK].astype(np.float16)
        return xi

    # ---- phase B ----
    ncB = _build_recur_nc(layout["plans_b"], nrhs_list, xslot,
                          want_t=True, want_y=False)
    S = sum(nrhs_list)
    offs = [sum(nrhs_list[:j]) for j in range(N_SLOTS)]
    nx = nrhs_list[xslot]

    def core_aux(core, nx_, particular_onehot):
        a = np.zeros((1, 4 * CHUNK + nx_), np.float16)
        if core == xcore:
            a[0, :4 * CHUNK] = x[:4 * CHUNK].astype(np.float16)
        if particular_onehot:
            a[0, 4 * CHUNK + nx_ - 1] = 1.0
        else:
            a[0, 4 * CHUNK:] = 1.0
        return a

    r0B = np.zeros((CHUNK, 4, S), np.float16)
    for j in range(N_SLOTS):
        r0B[:, :, offs[j]:offs[j] + nrhs_list[j]] = \
            _basis_ring0(nrhs_list[j] - 1)
    in_maps = [
        {"wts": _pack_wts(layout, layout["plans_b"], s),
         "aux": core_aux(s, nx, True), "ring0": r0B}
        for s in range(N_CORES)
    ]
    outsB = _run(ncB, in_maps, "phaseB")

    # ---- host combine (f64) ----
    seg_loc = {}
    for j in range(N_SLOTS):
        for i in range(N_CORES):
            seg_loc[slot_segs[j][i]] = (i, j)
    wins = [np.zeros(WIN, np.float64)]
    for seg in range(N_SEG):
        i, j = seg_loc[seg]
        basis = nrhs_list[j] - 1
        T = outsB[i]["tout"][:, :, offs[j]:offs[j] + basis + 1]
        T = T.astype(np.float64).transpose(1, 0, 2).reshape(WIN, basis + 1)
        w_next = T[:, :basis] @ wins[seg][WIN - basis:] + T[:, basis]
        wins.append(w_next)

    # ---- phase C: nrhs=1 with true initial windows ----
    ncC = _build_recur_nc(layout["plans_c"], [1] * N_SLOTS, xslot,
                          want_t=False, want_y=True)
    in_maps = []
    for s in range(N_CORES):
        r0 = np.zeros((CHUNK, 4, N_SLOTS), np.float16)
        for j in range(N_SLOTS):
            seg = slot_segs[j][s]
            w0 = wins[seg].astype(np.float16)
            r0[:, :, j] = w0.reshape(4, CHUNK).T
        in_maps.append({"wts": _pack_wts(layout, layout["plans_c"], s),
                        "aux": core_aux(s, 1, False), "ring0": r0})
    outsC = _run(ncC, in_maps, "phaseC")

    y = np.zeros(n, np.float32)
    for s in range(N_CORES):
        yo = outsC[s]["yout"]                        # [128, CPS*N_SLOTS]
        for j in range(N_SLOTS):
            seg = slot_segs[j][s]
            for c in range(CPS):
                y[(seg * CPS + c) * CHUNK:(seg * CPS + c + 1) * CHUNK] = \
                    yo[:, c * N_SLOTS + j]
    return y.astype(np.float32)
